# revision 21
# baseline (speedup 1.0000x reference)
"""Trainium2 Bass kernel for a 5x5 valid convolution over 96x96 images.

Reference computes x @ W.T where W is the [8464, 9216] conv-as-matmul
matrix (10 GFLOP dense).  We instead compute the convolution directly on
the tensor engine as 5 PSUM-accumulated banded matmuls (row-conv over the
image-row contraction, column shifts folded into the rhs access pattern):

    out[oi, b, oj] = sum_kj  B_kj.T @ X[:, b, oj+kj]
    B_kj[i, oi]    = K[i-oi, kj]   (banded Toeplitz, built on device)

Sharding: data-parallel over batch; each of the 8 cores convolves 8
images.  Raw Bass without a Block, hand-scheduled static DAG.  The B
build is pipelined per kj stripe (scatter taps -> banded load -> reverse)
across both HWDGE rings so the first matmul starts ~2.5us earlier than a
monolithic build; matmuls run kj-outer so each stripe is consumed as it
lands.
"""

import sys

sys.path.insert(0, "/opt/trn_rl_repo")

import numpy as np

import bass_rust
import concourse.bass as bass
import concourse.mybir as mybir
from concourse.bass_utils import run_bass_kernel_spmd

# Problem geometry (hardcoded per the task contract).
BATCH = 64
IN = 96           # input image side
KD = 5            # conv kernel side
OD = IN - KD + 1  # output side = 92
ISIZE = IN * IN   # 9216
OSIZE = OD * OD   # 8464
NCORES = 8
BPC = BATCH // NCORES  # images per core = 8
HALF = BPC // 2        # images per PSUM accumulation group = 4
QTR = BPC // 4         # images per store quarter = 2
UL = 187               # per-kj stripe length in the padded tap vector u


def _ap(view, offset, dims):
    ap = view.copy()
    ap.offset = offset
    ap.ap = bass_rust.VecI64Pair(dims)
    return ap


def _build_program():
    nc = bass.Bass()
    dt = mybir.dt.float32
    f32r = mybir.dt.float32r

    x_in = nc.declare_dram_parameter("x", [BPC, ISIZE], dt, isOutput=False)
    k_in = nc.declare_dram_parameter("k", [KD, KD], dt, isOutput=False)
    y_out = nc.declare_dram_parameter("y", [BPC, OSIZE], dt, isOutput=True)
    # Zero-initialized at NEFF load; per-run the scatters below overwrite
    # all 25 tap positions, so repeated executions stay correct.
    u_dram = nc.inline_tensor(np.zeros(KD * UL, np.float32), "u_scratch")

    from contextlib import ExitStack

    with ExitStack() as ctx:
        b_tmp = ctx.enter_context(nc.sbuf_tensor("b_tmp", [IN, KD, OD], dt))
        b_sb = ctx.enter_context(nc.sbuf_tensor("b_sb", [IN, KD, OD], f32r))
        x_sb = ctx.enter_context(nc.sbuf_tensor("x_sb", [IN, BPC, IN], dt))
        x_r = ctx.enter_context(nc.sbuf_tensor("x_r", [IN, BPC, IN], f32r))
        out_sb = ctx.enter_context(nc.sbuf_tensor("out_sb", [OD, BPC, OD], dt))
        ps0 = ctx.enter_context(nc.psum_tensor("ps0", [OD, HALF, OD], dt))
        ps1 = ctx.enter_context(nc.psum_tensor("ps1", [OD, HALF, OD], dt))
        sem = lambda n: ctx.enter_context(nc.semaphore(n))
        sem_x = sem("sem_x")          # x -> x_sb
        sem_xr = sem("sem_xr")        # x rounded to f32r
        sem_scat0 = sem("sem_scat0")  # taps of stripe 0
        sem_scatr = sem("sem_scatr")  # taps of stripes 1..4
        sem_bt = [sem(f"sem_bt{i}") for i in range(KD)]  # per-stripe loads
        sem_brev = sem("sem_brev")    # stripes reversed -> b_sb (1 per)
        sem_mm = sem("sem_mm")        # psum group done
        sem_copy = sem("sem_copy")    # psum -> out_sb quarter done
        sem_y = sem("sem_y")          # out_sb -> y

        psums = [ps0, ps1]

        def u_stripe(kj):
            return _ap(u_dram[:], kj * UL, [[1, IN], [1, OD]])

        # ---- scalar (ACT ring): tap scatters, then banded loads 1 & 3
        # u[kj*UL + 91 + t] = K[t, kj]; stripe 0 goes first so its banded
        # load can start while the remaining taps are still in flight.
        with nc.allow_non_contiguous_dma(reason="5-element tap scatter"):
            nc.scalar.dma_start(
                out=_ap(u_dram[:], OD - 1, [[UL, 1], [1, KD]]),
                in_=_ap(k_in[:], 0, [[1, 1], [KD, KD]]),
            ).then_inc(sem_scat0, 16)
            nc.scalar.dma_start(
                out=_ap(u_dram[:], UL + OD - 1, [[UL, KD - 1], [1, KD]]),
                in_=_ap(k_in[:], 1, [[1, KD - 1], [KD, KD]]),
            ).then_inc(sem_scatr, 16)

        # ---- sync (SP ring): x load first, then banded loads 0, 2, 4
        nc.sync.dma_start(
            out=x_sb[:],
            in_=_ap(x_in[:], 0, [[IN, IN], [ISIZE, BPC], [1, IN]]),
        ).then_inc(sem_x, 16)

        # B_tmp[p, kj, r] = u[kj*UL + p + r]  (= B[p, kj, 91-r])
        def btmp_load(engine, kj, sem, val):
            engine.wait_ge(sem, val)
            engine.dma_start(
                out=b_tmp[:, kj, :], in_=u_stripe(kj)
            ).then_inc(sem_bt[kj], 16)

        btmp_load(nc.sync, 0, sem_scat0, 16)
        btmp_load(nc.scalar, 1, sem_scatr, 16)
        btmp_load(nc.sync, 2, sem_scatr, 16)
        btmp_load(nc.scalar, 3, sem_scatr, 16)
        btmp_load(nc.sync, 4, sem_scatr, 16)

        # ---- vector: f32r rounding of x, per-stripe B reversal
        nc.vector.wait_ge(sem_x, 16)
        nc.vector.tensor_copy(x_r[:], x_sb[:]).then_inc(sem_xr, 1)
        for kj in range(KD):
            nc.vector.wait_ge(sem_bt[kj], 16)
            # reverse the oi axis: B[p, kj, oi] = B_tmp[p, kj, 91-oi]
            nc.vector.tensor_copy(
                b_sb[:, kj, :],
                _ap(b_tmp[:], kj * OD + OD - 1, [[KD * OD, IN], [-1, OD]]),
            ).then_inc(sem_brev, 1)

        # ---- tensor: kj-outer accumulated f32r matmuls, consume stripes
        nc.tensor.wait_ge(sem_xr, 1)
        for kj in range(KD):
            nc.tensor.wait_ge(sem_brev, kj + 1)
            for h in range(2):
                mm = nc.tensor.matmul(
                    psums[h][:],
                    b_sb[:, kj, :],
                    _ap(
                        x_r[:],
                        h * HALF * IN + kj,
                        [[BPC * IN, IN], [IN, HALF], [1, OD]],
                    ),
                    start=(kj == 0),
                    stop=(kj == KD - 1),
                )
                if kj == KD - 1:
                    mm.then_inc(sem_mm, 1)

        # ---- vector: quarter copies psum -> out_sb (q covers images 2q..2q+1)
        for q in range(4):
            h, lo = q // 2, (q % 2) * QTR
            nc.vector.wait_ge(sem_mm, h + 1)
            nc.vector.tensor_copy(
                out_sb[:, q * QTR : (q + 1) * QTR, :],
                psums[h][:, lo : lo + QTR, :],
            ).then_inc(sem_copy, 1)

        # ---- stores: quarters alternate between the two HWDGE rings
        def store(engine, q):
            engine.wait_ge(sem_copy, q + 1)
            engine.dma_start(
                out=_ap(
                    y_out[:],
                    q * QTR * OSIZE,
                    [[OD, OD], [OSIZE, QTR], [1, OD]],
                ),
                in_=out_sb[:, q * QTR : (q + 1) * QTR, :],
            ).then_inc(sem_y, 16)

        store(nc.sync, 0)
        store(nc.scalar, 1)
        store(nc.sync, 2)
        store(nc.scalar, 3)
        # hold execution open until every store has landed
        nc.sync.wait_ge(sem_y, 64)

    return nc


_NC = None


def kernel(x: np.ndarray, kernel: np.ndarray) -> np.ndarray:
    global _NC
    if _NC is None:
        _NC = _build_program()

    x = np.ascontiguousarray(x, dtype=np.float32)
    k = np.ascontiguousarray(kernel, dtype=np.float32)
    in_maps = [
        {"x": x[c * BPC : (c + 1) * BPC], "k": k} for c in range(NCORES)
    ]
    res = run_bass_kernel_spmd(_NC, in_maps, list(range(NCORES)))
    return np.concatenate([res.results[c]["y"] for c in range(NCORES)], axis=0)


# revision 23
# speedup vs baseline: 1.0136x; 1.0136x over previous
"""Trainium2 Bass kernel for a 5x5 valid convolution over 96x96 images.

Reference computes x @ W.T where W is the [8464, 9216] conv-as-matmul
matrix (10 GFLOP dense).  We instead compute the convolution directly on
the tensor engine as 5 PSUM-accumulated banded matmuls (row-conv over the
image-row contraction, column shifts folded into the rhs access pattern):

    out[oi, b, oj] = sum_kj  B_kj.T @ X[:, b, oj+kj]
    B_kj[i, oi]    = K[i-oi, kj]   (banded Toeplitz, built on device)

Sharding: data-parallel over batch; each of the 8 cores convolves 8
images.  Raw Bass without a Block, hand-scheduled static DAG.  The B
build is pipelined per kj stripe (scatter taps -> banded load -> reverse)
across both HWDGE rings so the first matmul starts ~2.5us earlier than a
monolithic build; matmuls run kj-outer so each stripe is consumed as it
lands.
"""

import sys

sys.path.insert(0, "/opt/trn_rl_repo")

import numpy as np

import bass_rust
import concourse.bass as bass
import concourse.mybir as mybir
from concourse.bass_utils import run_bass_kernel_spmd

# Problem geometry (hardcoded per the task contract).
BATCH = 64
IN = 96           # input image side
KD = 5            # conv kernel side
OD = IN - KD + 1  # output side = 92
ISIZE = IN * IN   # 9216
OSIZE = OD * OD   # 8464
NCORES = 8
BPC = BATCH // NCORES  # images per core = 8
HALF = BPC // 2        # images per PSUM accumulation group = 4
QTR = BPC // 4         # images per store quarter = 2
UL = 187               # per-kj stripe length in the padded tap vector u


def _ap(view, offset, dims):
    ap = view.copy()
    ap.offset = offset
    ap.ap = bass_rust.VecI64Pair(dims)
    return ap


def _build_program():
    nc = bass.Bass()
    dt = mybir.dt.float32
    f32r = mybir.dt.float32r

    x_in = nc.declare_dram_parameter("x", [BPC, ISIZE], dt, isOutput=False)
    k_in = nc.declare_dram_parameter("k", [KD, KD], dt, isOutput=False)
    y_out = nc.declare_dram_parameter("y", [BPC, OSIZE], dt, isOutput=True)
    # Zero-initialized at NEFF load; per-run the scatters below overwrite
    # all 25 tap positions, so repeated executions stay correct.
    u_dram = nc.inline_tensor(np.zeros(KD * UL, np.float32), "u_scratch")

    from contextlib import ExitStack

    with ExitStack() as ctx:
        b_tmp = ctx.enter_context(nc.sbuf_tensor("b_tmp", [IN, KD, OD], dt))
        b_sb = ctx.enter_context(nc.sbuf_tensor("b_sb", [IN, KD, OD], f32r))
        x_sb = ctx.enter_context(nc.sbuf_tensor("x_sb", [IN, BPC, IN], dt))
        x_r = ctx.enter_context(nc.sbuf_tensor("x_r", [IN, BPC, IN], f32r))
        out_sb = ctx.enter_context(nc.sbuf_tensor("out_sb", [OD, BPC, OD], dt))
        ps0 = ctx.enter_context(nc.psum_tensor("ps0", [OD, HALF, OD], dt))
        ps1 = ctx.enter_context(nc.psum_tensor("ps1", [OD, HALF, OD], dt))
        sem = lambda n: ctx.enter_context(nc.semaphore(n))
        sem_x = sem("sem_x")          # x -> x_sb
        sem_xr = sem("sem_xr")        # x rounded to f32r
        sem_scat0 = sem("sem_scat0")  # taps of stripe 0
        sem_scatr = sem("sem_scatr")  # taps of stripes 1..4
        sem_bt = [sem(f"sem_bt{i}") for i in range(KD)]  # per-stripe loads
        sem_brev = sem("sem_brev")    # stripes reversed -> b_sb (1 per)
        sem_mm = sem("sem_mm")        # psum group done
        sem_copy = sem("sem_copy")    # psum -> out_sb quarter done
        sem_y = sem("sem_y")          # out_sb -> y

        psums = [ps0, ps1]

        def u_stripe(kj):
            return _ap(u_dram[:], kj * UL, [[1, IN], [1, OD]])

        # u[kj*UL + 91 + t] = K[t, kj]; stripe 0's taps go first (on the
        # sync ring, whose first-DMA issue overhead is lower) so its
        # banded load can start while the remaining taps are in flight.
        with nc.allow_non_contiguous_dma(reason="5-element tap scatter"):
            nc.sync.dma_start(
                out=_ap(u_dram[:], OD - 1, [[UL, 1], [1, KD]]),
                in_=_ap(k_in[:], 0, [[1, 1], [KD, KD]]),
            ).then_inc(sem_scat0, 16)
            nc.scalar.dma_start(
                out=_ap(u_dram[:], UL + OD - 1, [[UL, KD - 1], [1, KD]]),
                in_=_ap(k_in[:], 1, [[1, KD - 1], [KD, KD]]),
            ).then_inc(sem_scatr, 16)

        # ---- sync (SP ring): x load, then banded loads 0, 2, 4
        nc.sync.dma_start(
            out=x_sb[:],
            in_=_ap(x_in[:], 0, [[IN, IN], [ISIZE, BPC], [1, IN]]),
        ).then_inc(sem_x, 16)

        # B_tmp[p, kj, r] = u[kj*UL + p + r]  (= B[p, kj, 91-r])
        def btmp_load(engine, kj, sem, val):
            engine.wait_ge(sem, val)
            engine.dma_start(
                out=b_tmp[:, kj, :], in_=u_stripe(kj)
            ).then_inc(sem_bt[kj], 16)

        btmp_load(nc.sync, 0, sem_scat0, 16)
        btmp_load(nc.scalar, 1, sem_scatr, 16)
        btmp_load(nc.sync, 2, sem_scatr, 16)
        btmp_load(nc.scalar, 3, sem_scatr, 16)
        btmp_load(nc.sync, 4, sem_scatr, 16)

        # ---- vector: f32r rounding of x, per-stripe B reversal
        nc.vector.wait_ge(sem_x, 16)
        nc.vector.tensor_copy(x_r[:], x_sb[:]).then_inc(sem_xr, 1)
        for kj in range(KD):
            nc.vector.wait_ge(sem_bt[kj], 16)
            # reverse the oi axis: B[p, kj, oi] = B_tmp[p, kj, 91-oi]
            nc.vector.tensor_copy(
                b_sb[:, kj, :],
                _ap(b_tmp[:], kj * OD + OD - 1, [[KD * OD, IN], [-1, OD]]),
            ).then_inc(sem_brev, 1)

        # ---- tensor: h-outer accumulated f32r matmuls; h0 consumes the
        # B stripes as they land, and finishes early so its stores can
        # overlap h1's matmuls.
        nc.tensor.wait_ge(sem_xr, 1)
        for h in range(2):
            for kj in range(KD):
                if h == 0:
                    nc.tensor.wait_ge(sem_brev, kj + 1)
                mm = nc.tensor.matmul(
                    psums[h][:],
                    b_sb[:, kj, :],
                    _ap(
                        x_r[:],
                        h * HALF * IN + kj,
                        [[BPC * IN, IN], [IN, HALF], [1, OD]],
                    ),
                    start=(kj == 0),
                    stop=(kj == KD - 1),
                )
                if kj == KD - 1:
                    mm.then_inc(sem_mm, 1)

        # ---- vector: quarter copies psum -> out_sb (q covers images 2q..2q+1)
        for q in range(4):
            h, lo = q // 2, (q % 2) * QTR
            nc.vector.wait_ge(sem_mm, h + 1)
            nc.vector.tensor_copy(
                out_sb[:, q * QTR : (q + 1) * QTR, :],
                psums[h][:, lo : lo + QTR, :],
            ).then_inc(sem_copy, 1)

        # ---- stores: quarters alternate between the two HWDGE rings
        def store(engine, q):
            engine.wait_ge(sem_copy, q + 1)
            engine.dma_start(
                out=_ap(
                    y_out[:],
                    q * QTR * OSIZE,
                    [[OD, OD], [OSIZE, QTR], [1, OD]],
                ),
                in_=out_sb[:, q * QTR : (q + 1) * QTR, :],
            ).then_inc(sem_y, 16)

        store(nc.sync, 0)
        store(nc.scalar, 1)
        store(nc.sync, 2)
        store(nc.scalar, 3)
        # hold execution open until every store has landed
        nc.sync.wait_ge(sem_y, 64)

    return nc


_NC = None


def kernel(x: np.ndarray, kernel: np.ndarray) -> np.ndarray:
    global _NC
    if _NC is None:
        _NC = _build_program()

    x = np.ascontiguousarray(x, dtype=np.float32)
    k = np.ascontiguousarray(kernel, dtype=np.float32)
    in_maps = [
        {"x": x[c * BPC : (c + 1) * BPC], "k": k} for c in range(NCORES)
    ]
    res = run_bass_kernel_spmd(_NC, in_maps, list(range(NCORES)))
    return np.concatenate([res.results[c]["y"] for c in range(NCORES)], axis=0)
